# revision 1
# baseline (speedup 1.0000x reference)
"""IrregularRNN (exact LTC cell) Trainium2 Bass kernel.

Strategy: tensor-parallel split of the 2U=2048 pre-activation columns
across 8 cores. Core k computes pre columns {f: [k*128,(k+1)*128),
a: [U+k*128, U+(k+1)*128)} for the FULL batch B=128 (full PE
utilization), updates h columns [k*128,(k+1)*128), transposes its
h'-slice on the PE, and AllGathers the transposed slices so every core
has the full h^T (as 8 ready-to-use lhsT K-chunks) for the next step's
h @ Wh matmul.  The x_t @ Wx part + bias only depend on inputs, so
those matmuls are issued ahead and hide inside the AllGather wait.

All layout transforms (transposes, weight slicing, broadcast of tau)
are done host-side in numpy; the device loop is 256 fully-unrolled
steps.
"""

import sys

sys.path.insert(0, "/opt/trn_rl_repo")

import numpy as np

B, T, D, U = 128, 256, 256, 1024
NC = 8
SL = U // NC          # h columns per core (128)
PW = 2 * SL           # pre-activation columns per core (256)
DK = D // 128         # K-chunks for x part (2)
UK = U // 128         # K-chunks for h part (8)

_CACHE: dict = {}


def _build(n_steps: int, use_collective: bool = True, repeat: int = 1):
    """Build + bacc-compile the SPMD Bass module for n_steps timesteps.

    repeat>1 is a timing-only mode: the T-loop body runs repeat times over
    the same inputs/outputs (numerically wrong; isolates on-device exec
    time from harness data-shipping via wall(2x) - wall(1x))."""
    import concourse.bacc as bacc
    import concourse.tile as tile
    from concourse import mybir

    AF = mybir.ActivationFunctionType
    f32 = mybir.dt.float32

    nc = bacc.Bacc(
        "TRN2",
        target_bir_lowering=False,
        debug=False,
        enable_asserts=False,
        num_devices=NC,
    )

    # --- kernel I/O ---------------------------------------------------
    xT = nc.dram_tensor("xT", [n_steps, DK, 128, B], f32, kind="ExternalInput")
    wx_sl = nc.dram_tensor("wx_sl", [DK, 128, PW], f32, kind="ExternalInput")
    wh_sl = nc.dram_tensor("wh_sl", [UK, 128, PW], f32, kind="ExternalInput")
    b_sl = nc.dram_tensor("b_sl", [1, PW], f32, kind="ExternalInput")
    tau_b = nc.dram_tensor("tau_b", [B, SL], f32, kind="ExternalInput")
    ndt = nc.dram_tensor("ndt", [B, n_steps], f32, kind="ExternalInput")
    h0T = nc.dram_tensor("h0T", [UK, 128, B], f32, kind="ExternalInput")
    h0_sl = nc.dram_tensor("h0_sl", [B, SL], f32, kind="ExternalInput")
    ones = nc.dram_tensor("ones", [1, 128], f32, kind="ExternalInput")
    ident = nc.dram_tensor("ident", [128, 128], f32, kind="ExternalInput")
    ys_sl = nc.dram_tensor("ys_sl", [n_steps, B, SL], f32, kind="ExternalOutput")

    RG = [list(range(NC))]

    with tile.TileContext(nc) as tc:
        with (
            tc.tile_pool(name="const", bufs=1) as cpool,
            tc.tile_pool(name="xin", bufs=6) as xpool,
            tc.tile_pool(name="hT", bufs=2) as hTpool,
            tc.tile_pool(name="act", bufs=3) as apool,
            tc.tile_pool(name="hnew", bufs=3) as hpool,
            tc.tile_pool(name="pre", bufs=2, space="PSUM") as prepool,
            tc.tile_pool(name="trp", bufs=2, space="PSUM") as trpool,
            tc.tile_pool(name="agio", bufs=2, space="DRAM") as dpool,
        ):
            # --- constants, loaded once -------------------------------
            wx_sb = cpool.tile([128, DK, PW], f32, name="wx_sb")
            nc.sync.dma_start(out=wx_sb[:], in_=wx_sl.ap().rearrange("c p n -> p c n"))
            wh_sb = cpool.tile([128, UK, PW], f32, name="wh_sb")
            nc.sync.dma_start(out=wh_sb[:], in_=wh_sl.ap().rearrange("c p n -> p c n"))
            b_sb = cpool.tile([1, PW], f32, name="b_sb")
            nc.sync.dma_start(out=b_sb[:], in_=b_sl[:])
            ones_sb = cpool.tile([1, 128], f32, name="ones_sb")
            nc.sync.dma_start(out=ones_sb[:], in_=ones[:])
            tau_sb = cpool.tile([B, SL], f32, name="tau_sb")
            nc.sync.dma_start(out=tau_sb[:], in_=tau_b[:])
            ndt_sb = cpool.tile([B, n_steps], f32, name="ndt_sb")
            nc.sync.dma_start(out=ndt_sb[:], in_=ndt[:])
            ident_sb = cpool.tile([128, 128], f32, name="ident_sb")
            nc.sync.dma_start(out=ident_sb[:], in_=ident[:])

            # initial state
            h_prev = hpool.tile([B, SL], f32, name="h_new")
            nc.sync.dma_start(out=h_prev[:], in_=h0_sl[:])
            hT_cur = []
            for j in range(UK):
                t_ = hTpool.tile([128, B], f32, name=f"hT{j}")
                nc.sync.dma_start(out=t_[:], in_=h0T[j])
                hT_cur.append(t_)

            # --- the recurrence ---------------------------------------
            for tv in range(n_steps * repeat):
                t = tv % n_steps
                # x_t^T chunks: [d-chunk partitions, batch free]
                xt = xpool.tile([128, DK, B], f32, name="xt")
                nc.sync.dma_start(out=xt[:], in_=xT[t].rearrange("c p b -> p c b"))

                pre = prepool.tile([B, PW], f32, name="pre")
                # bias + x part: no dependency on h -> runs during the
                # previous step's AllGather wait.
                nc.tensor.matmul(pre[:], ones_sb[:], b_sb[:], start=True, stop=False)
                for c in range(DK):
                    nc.tensor.matmul(
                        pre[:], xt[:, c, :], wx_sb[:, c, :], start=False, stop=False
                    )
                # h part
                for j in range(UK):
                    nc.tensor.matmul(
                        pre[:],
                        hT_cur[j][:],
                        wh_sb[:, j, :],
                        start=False,
                        stop=(j == UK - 1),
                    )

                # sigmoid(x) = 0.5 + 0.5*tanh(x/2): keeps every activation in
                # the exp_and_others ACT table set (sigmoid lives in a
                # different set; alternating sets costs ~2.7us per switch,
                # twice per step)
                f = apool.tile([B, SL], f32, name="f")
                nc.scalar.activation(f[:], pre[:, 0:SL], AF.Tanh, scale=0.5)
                a = apool.tile([B, SL], f32, name="a")
                nc.scalar.activation(a[:], pre[:, SL:PW], AF.Tanh)
                # g = tau + sigmoid(pre_f) = (tau + 0.5) + 0.5*tanh; tau_sb
                # holds tau + 0.5 (host-side)
                g = apool.tile([B, SL], f32, name="g")
                nc.vector.scalar_tensor_tensor(
                    g[:], f[:], 0.5, tau_sb[:],
                    mybir.AluOpType.mult, mybir.AluOpType.add,
                )
                dcy = apool.tile([B, SL], f32, name="dcy")
                nc.scalar.activation(
                    dcy[:], g[:], AF.Exp, scale=ndt_sb[:, t : t + 1]
                )
                hma = apool.tile([B, SL], f32, name="hma")
                nc.vector.tensor_sub(hma[:], h_prev[:], a[:])
                hd = apool.tile([B, SL], f32, name="hd")
                nc.vector.tensor_mul(hd[:], hma[:], dcy[:])
                h_new = hpool.tile([B, SL], f32, name="h_new")
                nc.vector.tensor_add(h_new[:], hd[:], a[:])

                nc.sync.dma_start(out=ys_sl[t], in_=h_new[:])

                if tv == n_steps * repeat - 1:
                    h_prev = h_new
                    break

                # h'^T slice for the next step's matmul
                trp = trpool.tile([128, B], f32, name="trp")
                nc.tensor.transpose(trp[:], h_new[:], ident_sb[:])
                trs = apool.tile([128, B], f32, name="trs")
                nc.vector.tensor_copy(trs[:], trp[:])
                ag_in = dpool.tile([128, B], f32, name="ag_in")
                nc.sync.dma_start(out=ag_in[:], in_=trs[:])
                if use_collective:
                    ag_out = dpool.tile(
                        [UK * 128, B], f32, name="ag_out", addr_space="Shared"
                    )
                    nc.gpsimd.collective_compute(
                        "AllGather",
                        mybir.AluOpType.bypass,
                        replica_groups=RG,
                        ins=[ag_in[:].opt()],
                        outs=[ag_out[:].opt()],
                    )
                hT_next = []
                for j in range(UK):
                    t_ = hTpool.tile([128, B], f32, name=f"hT{j}")
                    if use_collective:
                        nc.sync.dma_start(
                            out=t_[:], in_=ag_out[j * 128 : (j + 1) * 128, :]
                        )
                    else:
                        # timing-only bisect variant: local slice in place of
                        # the gathered one (numerically wrong on purpose)
                        nc.sync.dma_start(out=t_[:], in_=ag_in[:])
                    hT_next.append(t_)
                hT_cur = hT_next
                h_prev = h_new

    nc.compile()
    return nc


def _prep_inputs(features, time_steps, Wx, Wh, b, w_tau, h0, n_steps):
    """Host-side sharding + layout transforms -> per-core in_maps."""
    f32 = np.float32
    features = np.asarray(features, dtype=f32)
    time_steps = np.asarray(time_steps, dtype=f32)
    Wx = np.asarray(Wx, dtype=f32)
    Wh = np.asarray(Wh, dtype=f32)
    b = np.asarray(b, dtype=f32)
    w_tau = np.asarray(w_tau, dtype=f32)
    h0 = np.asarray(h0, dtype=f32)

    # softplus(w_tau), fp32
    tau = np.log1p(np.exp(w_tau)).astype(f32)

    xT = np.ascontiguousarray(features.transpose(1, 2, 0)).reshape(n_steps, DK, 128, B)
    ndt = np.ascontiguousarray(-time_steps)                      # [B, T]
    h0T = np.ascontiguousarray(h0.T).reshape(UK, 128, B)
    ones = np.ones((1, 128), dtype=f32)
    ident = np.eye(128, dtype=f32)

    in_maps = []
    for k in range(NC):
        cols = np.concatenate(
            [np.arange(k * SL, (k + 1) * SL), U + np.arange(k * SL, (k + 1) * SL)]
        )
        in_maps.append(
            {
                "xT": xT,
                "wx_sl": np.ascontiguousarray(Wx[:, cols]).reshape(DK, 128, PW),
                "wh_sl": np.ascontiguousarray(Wh[:, cols]).reshape(UK, 128, PW),
                "b_sl": np.ascontiguousarray(b[cols]).reshape(1, PW),
                "tau_b": np.ascontiguousarray(
                    np.broadcast_to(tau[k * SL : (k + 1) * SL] + 0.5, (B, SL))
                ),
                "ndt": ndt,
                "h0T": h0T,
                "h0_sl": np.ascontiguousarray(h0[:, k * SL : (k + 1) * SL]),
                "ones": ones,
                "ident": ident,
            }
        )
    return in_maps


def _assemble(results):
    """[T, B, SL] slices per core -> [B, T, U] full output."""
    ys = np.concatenate([r["ys_sl"] for r in results], axis=2)  # [T, B, U]
    return np.ascontiguousarray(ys.transpose(1, 0, 2))


def kernel(features, time_steps, Wx, Wh, b, w_tau, h0, _trace=False, _repeat=1):
    from concourse import bass_utils

    n_steps = features.shape[1]
    key = (n_steps, _repeat)
    if key not in _CACHE:
        _CACHE[key] = _build(n_steps, repeat=_repeat)
    nc = _CACHE[key]

    in_maps = _prep_inputs(features, time_steps, Wx, Wh, b, w_tau, h0, n_steps)
    try:
        res = bass_utils.run_bass_kernel_spmd(
            nc, in_maps, core_ids=list(range(NC)), trace=_trace
        )
    except ModuleNotFoundError:
        # no NTFF profiling hook in this container — run untraced
        res = bass_utils.run_bass_kernel_spmd(
            nc, in_maps, core_ids=list(range(NC)), trace=False
        )
    out = _assemble(res.results)
    if _trace:
        return out, res
    return out


if __name__ == "__main__":
    # smoke test with random data
    rng = np.random.default_rng(0)
    feats = rng.standard_normal((B, T, D), dtype=np.float32)
    ts = rng.random((B, T), dtype=np.float32)
    Wx = rng.standard_normal((D, 2 * U), dtype=np.float32) / np.sqrt(D)
    Wh = rng.standard_normal((U, 2 * U), dtype=np.float32) / np.sqrt(U)
    b = np.zeros((2 * U,), dtype=np.float32)
    w_tau = rng.random((U,), dtype=np.float32)
    h0 = np.zeros((B, U), dtype=np.float32)
    out = kernel(feats, ts, Wx, Wh, b, w_tau, h0)
    print("output", out.shape, out.dtype)



# revision 2
# speedup vs baseline: 8.9642x; 8.9642x over previous
"""IrregularRNN (exact LTC cell) Trainium2 Bass kernel.

Strategy: tensor-parallel split of the 2U=2048 pre-activation columns
across 8 cores. Core k computes pre columns {f: [k*128,(k+1)*128),
a: [U+k*128, U+(k+1)*128)} for the FULL batch B=128, updates h columns
[k*128,(k+1)*128), transposes its h'-slice on the PE, and AllGathers
the transposed slices so every core has the full h^T for the next
step's h @ Wh matmul.

This environment executes through an instruction-level simulator whose
wall cost is dominated by a fixed ~30-80us per instruction, so the
kernel minimizes per-step instruction count:
  - one tanh over the whole [B, 2*SL] pre tile (sigmoid(x) =
    0.5 + 0.5*tanh(x/2); the 0.5 input scale is folded into the
    f-columns of Wx/Wh host-side)
  - no bias matmul when b == 0 (the reference uses b = zeros); a
    one-instruction DVE add fallback otherwise
  - ACT-engine copy PSUM->SBUF after the transpose (same act table set
    as Tanh/Exp, no table switch)
  - ONE strided DMA to load the gathered h^T as [128, 8, B] instead of
    8 per-chunk DMAs
  - ys stores batched: h' accumulates in an SBUF ring [B, 8, SL] and is
    flushed to DRAM once per 8 steps
"""

import sys

sys.path.insert(0, "/opt/trn_rl_repo")

import numpy as np

B, T, D, U = 128, 256, 256, 1024
NC = 8
SL = U // NC          # h columns per core (128)
PW = 2 * SL           # pre-activation columns per core (256)
DK = D // 128         # K-chunks for x part (2)
UK = U // 128         # K-chunks for h part (8)
YS_BATCH = 8          # steps of h' per ys DMA flush

_CACHE: dict = {}


def _build(n_steps: int, with_bias: bool = False, repeat: int = 1):
    """Build + bacc-compile the SPMD Bass module for n_steps timesteps.

    repeat>1 is a timing-only mode: the T-loop body runs repeat times over
    the same inputs/outputs (numerically wrong; isolates on-device exec
    time from harness data-shipping via wall(2x) - wall(1x))."""
    import concourse.bacc as bacc
    import concourse.tile as tile
    from concourse import mybir

    AF = mybir.ActivationFunctionType
    f32 = mybir.dt.float32

    nc = bacc.Bacc(
        "TRN2",
        target_bir_lowering=False,
        debug=False,
        enable_asserts=False,
        num_devices=NC,
    )

    # --- kernel I/O ---------------------------------------------------
    xT = nc.dram_tensor("xT", [n_steps, DK, 128, B], f32, kind="ExternalInput")
    wx_sl = nc.dram_tensor("wx_sl", [DK, 128, PW], f32, kind="ExternalInput")
    wh_sl = nc.dram_tensor("wh_sl", [UK, 128, PW], f32, kind="ExternalInput")
    b_sl = nc.dram_tensor("b_sl", [B, PW], f32, kind="ExternalInput")
    tau_b = nc.dram_tensor("tau_b", [B, SL], f32, kind="ExternalInput")
    ndt = nc.dram_tensor("ndt", [B, n_steps], f32, kind="ExternalInput")
    h0T = nc.dram_tensor("h0T", [UK, 128, B], f32, kind="ExternalInput")
    h0_sl = nc.dram_tensor("h0_sl", [B, SL], f32, kind="ExternalInput")
    ident = nc.dram_tensor("ident", [128, 128], f32, kind="ExternalInput")
    ys_sl = nc.dram_tensor("ys_sl", [n_steps, B, SL], f32, kind="ExternalOutput")

    RG = [list(range(NC))]

    with tile.TileContext(nc) as tc:
        with (
            tc.tile_pool(name="const", bufs=1) as cpool,
            tc.tile_pool(name="xin", bufs=6) as xpool,
            tc.tile_pool(name="hT", bufs=2) as hTpool,
            tc.tile_pool(name="act", bufs=3) as apool,
            tc.tile_pool(name="hbig", bufs=2) as hbpool,
            tc.tile_pool(name="pre", bufs=2, space="PSUM") as prepool,
            tc.tile_pool(name="trp", bufs=2, space="PSUM") as trpool,
            tc.tile_pool(name="agio", bufs=2, space="DRAM") as dpool,
        ):
            # --- constants, loaded once -------------------------------
            wx_sb = cpool.tile([128, DK, PW], f32, name="wx_sb")
            nc.sync.dma_start(out=wx_sb[:], in_=wx_sl.ap().rearrange("c p n -> p c n"))
            wh_sb = cpool.tile([128, UK, PW], f32, name="wh_sb")
            nc.sync.dma_start(out=wh_sb[:], in_=wh_sl.ap().rearrange("c p n -> p c n"))
            tau_sb = cpool.tile([B, SL], f32, name="tau_sb")
            nc.sync.dma_start(out=tau_sb[:], in_=tau_b[:])
            ndt_sb = cpool.tile([B, n_steps], f32, name="ndt_sb")
            nc.sync.dma_start(out=ndt_sb[:], in_=ndt[:])
            ident_sb = cpool.tile([128, 128], f32, name="ident_sb")
            nc.sync.dma_start(out=ident_sb[:], in_=ident[:])
            if with_bias:
                b_sb = cpool.tile([B, PW], f32, name="b_sb")
                nc.sync.dma_start(out=b_sb[:], in_=b_sl[:])

            # initial state
            h_prev = cpool.tile([B, SL], f32, name="h0_sb")
            nc.sync.dma_start(out=h_prev[:], in_=h0_sl[:])
            hT_cur = cpool.tile([128, UK, B], f32, name="hT0_sb")
            nc.sync.dma_start(out=hT_cur[:], in_=h0T.ap().rearrange("c p b -> p c b"))

            hbig = None
            # --- the recurrence ---------------------------------------
            for tv in range(n_steps * repeat):
                t = tv % n_steps
                # x_t^T chunks: [d-chunk partitions, batch free]
                xt = xpool.tile([128, DK, B], f32, name="xt")
                nc.sync.dma_start(out=xt[:], in_=xT[t].rearrange("c p b -> p c b"))

                pre = prepool.tile([B, PW], f32, name="pre")
                # x part first: no dependency on h -> can start while the
                # previous step's AllGather is still in flight.
                for c in range(DK):
                    nc.tensor.matmul(
                        pre[:], xt[:, c, :], wx_sb[:, c, :],
                        start=(c == 0), stop=False,
                    )
                for j in range(UK):
                    nc.tensor.matmul(
                        pre[:],
                        hT_cur[:, j, :],
                        wh_sb[:, j, :],
                        start=False,
                        stop=(j == UK - 1),
                    )

                if with_bias:
                    nc.vector.tensor_add(pre[:], pre[:], b_sb[:])

                # one tanh over the whole pre tile:
                #   cols [0,SL)  = tanh(0.5*pre_f)  (0.5 folded into weights)
                #   cols [SL,PW) = tanh(pre_a) = a
                ta = apool.tile([B, PW], f32, name="ta")
                nc.scalar.activation(ta[:], pre[:], AF.Tanh)
                a = ta[:, SL:PW]
                # g = tau + sigmoid(pre_f) = (tau + 0.5) + 0.5*tanh(pre_f/2)
                g = apool.tile([B, SL], f32, name="g")
                nc.vector.scalar_tensor_tensor(
                    g[:], ta[:, 0:SL], 0.5, tau_sb[:],
                    mybir.AluOpType.mult, mybir.AluOpType.add,
                )
                dcy = apool.tile([B, SL], f32, name="dcy")
                nc.scalar.activation(
                    dcy[:], g[:], AF.Exp, scale=ndt_sb[:, t : t + 1]
                )
                hma = apool.tile([B, SL], f32, name="hma")
                nc.vector.tensor_sub(hma[:], h_prev[:], a)
                hd = apool.tile([B, SL], f32, name="hd")
                nc.vector.tensor_mul(hd[:], hma[:], dcy[:])
                if t % YS_BATCH == 0:
                    hbig = hbpool.tile([B, YS_BATCH, SL], f32, name="hbig")
                h_new = hbig[:, t % YS_BATCH, :]
                nc.vector.tensor_add(h_new, hd[:], a)

                if t % YS_BATCH == YS_BATCH - 1:
                    # flush YS_BATCH steps of h' in one strided DMA
                    t0 = t - (YS_BATCH - 1)
                    nc.sync.dma_start(
                        out=ys_sl[t0 : t + 1].rearrange("s b u -> b s u"),
                        in_=hbig[:],
                    )

                if tv == n_steps * repeat - 1:
                    h_prev = h_new
                    break

                # h'^T slice for the next step's matmul
                trp = trpool.tile([128, B], f32, name="trp")
                nc.tensor.transpose(trp[:], h_new, ident_sb[:])
                trs = apool.tile([128, B], f32, name="trs")
                nc.scalar.activation(trs[:], trp[:], AF.Copy)
                ag_in = dpool.tile([128, B], f32, name="ag_in")
                nc.sync.dma_start(out=ag_in[:], in_=trs[:])
                ag_out = dpool.tile(
                    [UK * 128, B], f32, name="ag_out", addr_space="Shared"
                )
                nc.gpsimd.collective_compute(
                    "AllGather",
                    mybir.AluOpType.bypass,
                    replica_groups=RG,
                    ins=[ag_in[:].opt()],
                    outs=[ag_out[:].opt()],
                )
                hT_next = hTpool.tile([128, UK, B], f32, name="hTg")
                nc.sync.dma_start(
                    out=hT_next[:],
                    in_=ag_out[:].rearrange("(c p) b -> p c b", p=128),
                )
                hT_cur = hT_next
                h_prev = h_new

    nc.compile()
    return nc


def _prep_inputs(features, time_steps, Wx, Wh, b, w_tau, h0, n_steps):
    """Host-side sharding + layout transforms -> per-core in_maps."""
    f32 = np.float32
    features = np.asarray(features, dtype=f32)
    time_steps = np.asarray(time_steps, dtype=f32)
    Wx = np.asarray(Wx, dtype=f32)
    Wh = np.asarray(Wh, dtype=f32)
    b = np.asarray(b, dtype=f32)
    w_tau = np.asarray(w_tau, dtype=f32)
    h0 = np.asarray(h0, dtype=f32)

    # softplus(w_tau), fp32
    tau = np.log1p(np.exp(w_tau)).astype(f32)

    # fold the sigmoid half-angle scale into the f columns
    Wxs = Wx.copy()
    Wxs[:, :U] *= 0.5
    Whs = Wh.copy()
    Whs[:, :U] *= 0.5
    bs = b.copy()
    bs[:U] *= 0.5
    with_bias = bool(np.any(b != 0.0))

    xT = np.ascontiguousarray(features.transpose(1, 2, 0)).reshape(n_steps, DK, 128, B)
    ndt = np.ascontiguousarray(-time_steps)                      # [B, T]
    h0T = np.ascontiguousarray(h0.T).reshape(UK, 128, B)
    ident = np.eye(128, dtype=f32)

    in_maps = []
    for k in range(NC):
        cols = np.concatenate(
            [np.arange(k * SL, (k + 1) * SL), U + np.arange(k * SL, (k + 1) * SL)]
        )
        in_maps.append(
            {
                "xT": xT,
                "wx_sl": np.ascontiguousarray(Wxs[:, cols]).reshape(DK, 128, PW),
                "wh_sl": np.ascontiguousarray(Whs[:, cols]).reshape(UK, 128, PW),
                "b_sl": np.ascontiguousarray(
                    np.broadcast_to(bs[cols], (B, PW))
                ),
                "tau_b": np.ascontiguousarray(
                    np.broadcast_to(tau[k * SL : (k + 1) * SL] + 0.5, (B, SL))
                ),
                "ndt": ndt,
                "h0T": h0T,
                "h0_sl": np.ascontiguousarray(h0[:, k * SL : (k + 1) * SL]),
                "ident": ident,
            }
        )
    return in_maps, with_bias


def _assemble(results):
    """[T, B, SL] slices per core -> [B, T, U] full output."""
    ys = np.concatenate([r["ys_sl"] for r in results], axis=2)  # [T, B, U]
    return np.ascontiguousarray(ys.transpose(1, 0, 2))


def kernel(features, time_steps, Wx, Wh, b, w_tau, h0, _trace=False, _repeat=1):
    from concourse import bass_utils

    n_steps = features.shape[1]
    in_maps, with_bias = _prep_inputs(
        features, time_steps, Wx, Wh, b, w_tau, h0, n_steps
    )
    key = (n_steps, with_bias, _repeat)
    if key not in _CACHE:
        _CACHE[key] = _build(n_steps, with_bias=with_bias, repeat=_repeat)
    nc = _CACHE[key]

    try:
        res = bass_utils.run_bass_kernel_spmd(
            nc, in_maps, core_ids=list(range(NC)), trace=_trace
        )
    except ModuleNotFoundError:
        # no NTFF profiling hook in this container — run untraced
        res = bass_utils.run_bass_kernel_spmd(
            nc, in_maps, core_ids=list(range(NC)), trace=False
        )
    out = _assemble(res.results)
    if _trace:
        return out, res
    return out


if __name__ == "__main__":
    # smoke test with random data
    rng = np.random.default_rng(0)
    feats = rng.standard_normal((B, T, D), dtype=np.float32)
    ts = rng.random((B, T), dtype=np.float32)
    Wx = rng.standard_normal((D, 2 * U), dtype=np.float32) / np.sqrt(D)
    Wh = rng.standard_normal((U, 2 * U), dtype=np.float32) / np.sqrt(U)
    b = np.zeros((2 * U,), dtype=np.float32)
    w_tau = rng.random((U,), dtype=np.float32)
    h0 = np.zeros((B, U), dtype=np.float32)
    out = kernel(feats, ts, Wx, Wh, b, w_tau, h0)
    print("output", out.shape, out.dtype)
